# revision 8
# baseline (speedup 1.0000x reference)
"""Trainium2 Bass kernel for a 2-layer LSTM (B=256, T=512, I=64, H=256) + linear head.

Strategy (hardcoded, self-contained):
  - Data-parallel over batch across 8 NeuronCores (32 batch elems per core).
  - Per core, both LSTM layers run step-by-step in a feature-blocked layout:
      gate PSUM tile [128=(hblk4, b32), 256=(gate4, hh2, hl32)]
    produced by col-group-packed bf16 matmuls (tile_position=(0, 32*m)) that
    share the small transposed-state stationary hT [k, 32].
  - All matmul operands are bf16 (4x faster PE streaming than fp32; PSUM
    accumulation stays fp32).
  - All gate nonlinearities collapse into ONE scalar-engine tanh per
    layer-step via sigma(x) = (tanh(x/2)+1)/2; the 1/2 pre-scales for the
    i,f,o gate columns are folded into the weights host-side, and the
    resulting "+1 then scale" post-ops ride fused scalar_tensor_tensor DVE
    instructions. The cell state is carried as cs = 2c (fp32); hidden state
    as h2 = 2h (bf16) with the compensating 0.5 folded into every weight
    that consumes h.
  - Input projection x@Wih.T and all biases ride the same PSUM accumulation
    (augmented ones-row trick), so there is no separate projection pass.
  - A single DVE 32x32 block-transpose per layer-step turns h2 back into the
    next step's stationary hT.
  - The two output linear layers have no nonlinearity between them and are
    folded host-side into a single [256, 4] matmul + bias.
  - All weights ship as ONE packed DRAM blob -> one DMA -> one HWDGE queue
    semaphore, keeping per-instruction sync-wait counts within HW limits.
"""

import numpy as np

B, T, I, H, O = 256, 512, 64, 256, 4
NCORES = 8
BS = B // NCORES  # 32

# reference gate order is (i, f, g, o); we reorder to (i, f, o, g) so that the
# sigmoid-family gates are contiguous (cols 0:192) and the tanh gate is cols
# 192:256 of each 256-col group.
GATE_PERM = [0, 1, 3, 2]

# weight blob column offsets (bf16 elements, [128, WB_COLS])
OFF_W0 = 0        # Whh0 perm  [128, 2*1024]
OFF_W1 = 2048     # Whh1 perm  [128, 2*1024]
OFF_WX1 = 4096    # Wih1 perm  [128, 2*1024]
OFF_WX0 = 6144    # Wih0 perm + bias row, rows 0:65, [65, 1024]
OFF_B1 = 7168     # bias1 row, row 0, [1, 1024]
OFF_WF = 8192     # folded head weight [128, 2*4]
OFF_BF = 8200     # folded head bias, row 0, [1, 4]
OFF_XT = 8224     # x transposed + ones row, rows 0:65, [65, t_steps*32]
def _wb_cols(t_steps):
    return OFF_XT + t_steps * BS

_CACHED = {}


def _perm_cols(Wt):
    """Permute gate columns of [K, 1024] (col j = gate_orig*256 + h) into
    col = m*256 + gate_new*64 + hh*32 + hl, where h = hh*128 + m*32 + hl."""
    K = Wt.shape[0]
    W = Wt.reshape(K, 4, 256)[:, GATE_PERM, :]      # [K, gate, h]
    W = W.reshape(K, 4, 2, 4, 32)                    # [K, gate, hh, m, hl]
    W = W.transpose(0, 3, 1, 2, 4)                   # [K, m, gate, hh, hl]
    return np.ascontiguousarray(W.reshape(K, 1024), dtype=np.float32)


def _scale_gates(Wt):
    """Scale the i, f, o gate columns of [K, 1024] (ORIGINAL gate layout,
    col j = gate_orig*256 + h; orig order i,f,g,o) by 0.5 so that
    tanh(pre) == tanh(orig_pre/2) and sigma(orig_pre) = (tanh+1)/2."""
    s = np.ones(1024, np.float32)
    s[0:256] = 0.5      # i
    s[256:512] = 0.5    # f
    s[768:1024] = 0.5   # o
    return Wt * s[None, :]


def _build_bass(t_steps=T):
    import concourse.mybir as mybir
    import concourse.tile as tile
    from concourse import bacc
    from contextlib import ExitStack

    f32 = mybir.dt.float32
    bf16 = mybir.dt.bfloat16
    AF = mybir.ActivationFunctionType
    ALU = mybir.AluOpType

    nc = bacc.Bacc("TRN2", target_bir_lowering=False)

    wb_cols = _wb_cols(t_steps)
    wb_d = nc.dram_tensor("wb", (128, wb_cols), bf16, kind="ExternalInput")
    y_d = nc.dram_tensor("y", (BS, O), f32, kind="ExternalOutput")

    with tile.TileContext(nc) as tc, ExitStack() as ctx:
        const = ctx.enter_context(tc.tile_pool(name="const", bufs=1))
        state = ctx.enter_context(tc.tile_pool(name="state", bufs=1))
        work = ctx.enter_context(tc.tile_pool(name="work", bufs=3))
        hts = ctx.enter_context(tc.tile_pool(name="hts", bufs=4))
        psum = ctx.enter_context(tc.tile_pool(name="psum", bufs=3, space="PSUM"))

        wb = const.tile([128, wb_cols], bf16)
        nc.sync.dma_start(wb[:], wb_d[:])

        def xt_ap(t):
            return wb[0:65, OFF_XT + BS * t : OFF_XT + BS * t + BS]

        def w0_ap(kc, m):
            return wb[:, OFF_W0 + 1024 * kc + 256 * m : OFF_W0 + 1024 * kc + 256 * m + 256]

        def w1_ap(kc, m):
            return wb[:, OFF_W1 + 1024 * kc + 256 * m : OFF_W1 + 1024 * kc + 256 * m + 256]

        def wx1_ap(kc, m):
            return wb[:, OFF_WX1 + 1024 * kc + 256 * m : OFF_WX1 + 1024 * kc + 256 * m + 256]

        def wx0_ap(m):
            return wb[0:65, OFF_WX0 + 256 * m : OFF_WX0 + 256 * m + 256]

        def b1_ap(m):
            return wb[0:1, OFF_B1 + 256 * m : OFF_B1 + 256 * m + 256]

        cs0 = state.tile([128, 64], f32)
        cs1 = state.tile([128, 64], f32)
        nc.vector.memset(cs0[:], 0.0)
        nc.vector.memset(cs1[:], 0.0)
        hT0 = hts.tile([128, 64], bf16, tag="ht0")
        hT1 = hts.tile([128, 64], bf16, tag="ht1")
        nc.vector.memset(hT0[:], 0.0)
        nc.vector.memset(hT1[:], 0.0)
        ones_t = const.tile([1, BS], bf16)
        nc.vector.memset(ones_t[:], 1.0)
        ones_ap = ones_t[:]

        # HAM warm-up: ~40 back-to-back matmuls (~4.5us sustained PE activity
        # while the wb DMA is still in flight) flip the PE clock gate from
        # 4/8 (1.2 GHz) to 8/8 (2.4 GHz). The kernel's steady-state PE gaps
        # are well under the MID re-throttle window, so the PE stays warm for
        # the rest of the kernel.
        warm_src = const.tile([1, 256], bf16)
        nc.vector.memset(warm_src[:], 0.0)
        warm_ps = psum.tile([BS, 256], f32, tag="warm", bufs=1)
        for _ in range(40):
            nc.tensor.matmul(warm_ps[:], ones_ap, warm_src[:],
                             start=True, stop=True, skip_group_check=True)

        def elementwise(g, cs, tagsuf):
            # gate tile t holds tanh of (pre-scaled) gates: cols
            # 0:64=t_i, 64:128=t_f, 128:192=t_o, 192:256=t_g (per m-group)
            t = work.tile([128, 256], bf16, tag="t" + tagsuf)
            nc.scalar.activation(t[:], g[:], AF.Tanh)
            # A = (t_f + 1) * cs            ( = 4*sigma(f)*c )
            A = work.tile([128, 64], f32, tag="A" + tagsuf)
            nc.vector.scalar_tensor_tensor(
                A[:], t[:, 64:128], 1.0, cs[:], ALU.add, ALU.mult)
            # B = (t_i + 1) * t_g           ( = 2*sigma(i)*gtilde )
            Bt = work.tile([128, 64], bf16, tag="B" + tagsuf)
            nc.vector.scalar_tensor_tensor(
                Bt[:], t[:, 0:64], 1.0, t[:, 192:256], ALU.add, ALU.mult)
            # cs' = A*0.5 + B               ( = 2*c' )
            nc.vector.scalar_tensor_tensor(
                cs[:], A[:], 0.5, Bt[:], ALU.mult, ALU.add)
            # th = tanh(cs'/2) = tanh(c')
            th = work.tile([128, 64], bf16, tag="th" + tagsuf)
            nc.scalar.activation(th[:], cs[:], AF.Tanh, scale=0.5)
            # h2 = (t_o + 1) * th           ( = 2*h' )
            h2 = work.tile([128, 64], bf16, tag="h" + tagsuf)
            nc.vector.scalar_tensor_tensor(
                h2[:], t[:, 128:192], 1.0, th[:], ALU.add, ALU.mult)
            hT = hts.tile([128, 64], bf16, tag="ht" + tagsuf)
            nc.vector.transpose(hT[:], h2[:])
            return hT

        # --- matmul emission, split so dependency-free groups can be hoisted
        # ahead in PE FIFO order to fill chain-stall gaps ---
        g0_q = {}
        g1_q = {}

        def grp(g, lhsT, w_ap_m, start, stop):
            for m in range(4):
                nc.tensor.matmul(
                    g[32 * m : 32 * m + 32, :], lhsT, w_ap_m(m),
                    start=start, stop=stop,
                    tile_position=(0, 32 * m), skip_group_check=True,
                )

        def prefill0(t):
            g = psum.tile([128, 256], f32, tag="g0")
            grp(g, xt_ap(t), wx0_ap, True, False)
            g0_q[t] = g

        def prefill1(t):
            g = psum.tile([128, 256], f32, tag="g1")
            grp(g, ones_ap, b1_ap, True, False)
            g1_q[t] = g

        def add_h1(t, hT1_prev):
            g = g1_q[t]
            for kc in range(2):
                grp(g, hT1_prev[:, 32 * kc : 32 * kc + 32],
                    lambda m, kc=kc: w1_ap(kc, m), False, False)

        def finish0(t, hT0_prev):
            g = g0_q.pop(t)
            for kc in range(2):
                grp(g, hT0_prev[:, 32 * kc : 32 * kc + 32],
                    lambda m, kc=kc: w0_ap(kc, m), False, kc == 1)
            return elementwise(g, cs0, "0")

        def finish1(t, hT0_t):
            g = g1_q.pop(t)
            for kc in range(2):
                grp(g, hT0_t[:, 32 * kc : 32 * kc + 32],
                    lambda m, kc=kc: wx1_ap(kc, m), False, kc == 1)
            return elementwise(g, cs1, "1")

        prefill0(0)
        prefill0(1)
        prefill1(0)
        hT0_hist = [hT0]
        for t in range(t_steps):
            if t + 2 < t_steps:
                prefill0(t + 2)
            if t + 1 < t_steps:
                prefill1(t + 1)
            # hT1-side groups of layer-1 step t-1 are ready a full period
            # early; emit them before the chain-critical hT0-dependent groups.
            if t >= 1:
                add_h1(t - 1, hT1)
            hT0_new = finish0(t, hT0_hist[-1])
            hT0_hist.append(hT0_new)
            # layer 1 lags by one step so the two chains overlap
            if t >= 1:
                hT1 = finish1(t - 1, hT0_hist[-2])
            if len(hT0_hist) > 3:
                hT0_hist.pop(0)
        add_h1(t_steps - 1, hT1)
        hT1 = finish1(t_steps - 1, hT0_hist[-1])

        yp = psum.tile([BS, O], f32, tag="yh", bufs=1)
        nc.tensor.matmul(yp[:], ones_ap, wb[0:1, OFF_BF : OFF_BF + O], start=True, stop=False)
        nc.tensor.matmul(yp[:], hT1[:, 0:32], wb[:, OFF_WF : OFF_WF + O], start=False, stop=False)
        nc.tensor.matmul(yp[:], hT1[:, 32:64], wb[:, OFF_WF + O : OFF_WF + 2 * O], start=False, stop=True)
        y_sb = work.tile([BS, O], f32, tag="y")
        nc.vector.tensor_copy(y_sb[:], yp[:])
        nc.sync.dma_start(y_d[:], y_sb[:])

    return nc


def _prep_inputs(x, Wih0, Whh0, bih0, bhh0, Wih1, Whh1, bih1, bhh1, W1, b1, W2, b2,
                 t_steps=T):
    import ml_dtypes

    x = np.asarray(x, dtype=np.float32)[:, :t_steps, :]
    wb = np.zeros((128, _wb_cols(t_steps)), np.float32)
    # 0.5 on i,f,o gate cols (tanh-only activation); 0.5 on every weight row
    # that consumes h2 = 2h (Whh0, Whh1, Wih1, head Wf).
    wb[:, OFF_W0 : OFF_W0 + 2048] = _perm_cols(
        _scale_gates(np.asarray(Whh0, np.float32).T) * 0.5
    ).reshape(2, 128, 1024).transpose(1, 0, 2).reshape(128, 2048)
    wb[:, OFF_W1 : OFF_W1 + 2048] = _perm_cols(
        _scale_gates(np.asarray(Whh1, np.float32).T) * 0.5
    ).reshape(2, 128, 1024).transpose(1, 0, 2).reshape(128, 2048)
    wb[:, OFF_WX1 : OFF_WX1 + 2048] = _perm_cols(
        _scale_gates(np.asarray(Wih1, np.float32).T) * 0.5
    ).reshape(2, 128, 1024).transpose(1, 0, 2).reshape(128, 2048)
    wb[0:64, OFF_WX0 : OFF_WX0 + 1024] = _perm_cols(
        _scale_gates(np.asarray(Wih0, np.float32).T))
    wb[64, OFF_WX0 : OFF_WX0 + 1024] = _perm_cols(_scale_gates(
        (np.asarray(bih0, np.float32) + np.asarray(bhh0, np.float32))[None, :]))[0]
    wb[0, OFF_B1 : OFF_B1 + 1024] = _perm_cols(_scale_gates(
        (np.asarray(bih1, np.float32) + np.asarray(bhh1, np.float32))[None, :]))[0]
    Wf = (np.asarray(W1, np.float32).T @ np.asarray(W2, np.float32).T).astype(
        np.float32) * 0.5
    wb[:, OFF_WF : OFF_WF + 2 * O] = Wf.reshape(2, 128, O).transpose(1, 0, 2).reshape(128, 2 * O)
    wb[0, OFF_BF : OFF_BF + O] = (
        np.asarray(b1, np.float32) @ np.asarray(W2, np.float32).T + np.asarray(b2, np.float32))

    in_maps = []
    for c in range(NCORES):
        xc = x[c * BS : (c + 1) * BS]                       # [BS, t, I]
        xt = xc.transpose(2, 1, 0).reshape(I, t_steps * BS) # [I, t*BS]
        wbc = wb.copy()
        wbc[0:64, OFF_XT:] = xt
        wbc[64, OFF_XT:] = 1.0
        in_maps.append(dict(wb=wbc.astype(ml_dtypes.bfloat16)))
    return in_maps


def run(t_steps=T, trace=False, **inputs):
    from concourse.bass_utils import run_bass_kernel_spmd

    key = t_steps
    if key not in _CACHED:
        nc_new = _build_bass(t_steps)
        # finalize BEFORE handing to the PJRT path: the bass_exec lowering
        # otherwise finalizes with the partition-id register preamble in a
        # state that miscompiles (walrus "Reg has not been allocated yet")
        nc_new.finalize()
        _CACHED[key] = nc_new
    nc = _CACHED[key]
    in_maps = _prep_inputs(**inputs, t_steps=t_steps)
    res = None
    for attempt in range(4):
        try:
            res = run_bass_kernel_spmd(nc, in_maps, core_ids=list(range(NCORES)),
                                       trace=trace)
            break
        except Exception as e:  # flaky parallel-birverifier race in neuronx-cc
            if attempt == 3:
                raise
            print(f"run attempt {attempt} failed ({type(e).__name__}); retrying")
    assert res is not None
    y = np.concatenate([r["y"] for r in res.results], axis=0)
    return y, res


def kernel(**inputs):
    y, _ = run(t_steps=T, trace=False, **inputs)
    return y


# revision 10
# speedup vs baseline: 1.1498x; 1.1498x over previous
"""Trainium2 Bass kernel for a 2-layer LSTM (B=256, T=512, I=64, H=256) + linear head.

Strategy (hardcoded, self-contained):
  - Data-parallel over batch across 8 NeuronCores (32 batch elems per core).
  - Per core, both LSTM layers run step-by-step in a feature-blocked layout:
      gate PSUM tile [128=(hblk4, b32), 256=(gate4, hh2, hl32)]
    produced by col-group-packed bf16 matmuls (tile_position=(0, 32*m)) that
    share the small transposed-state stationary hT [k, 32].
  - All matmul operands are bf16 (4x faster PE streaming than fp32; PSUM
    accumulation stays fp32).
  - All gate nonlinearities collapse into ONE scalar-engine tanh per
    layer-step via sigma(x) = (tanh(x/2)+1)/2; the 1/2 pre-scales for the
    i,f,o gate columns are folded into the weights host-side, and the
    resulting "+1 then scale" post-ops ride fused scalar_tensor_tensor DVE
    instructions. The cell state is carried as cs = 2c (fp32); hidden state
    as h2 = 2h (bf16) with the compensating 0.5 folded into every weight
    that consumes h.
  - Input projection x@Wih.T and all biases ride the same PSUM accumulation
    (augmented ones-row trick), so there is no separate projection pass.
  - A single DVE 32x32 block-transpose per layer-step turns h2 back into the
    next step's stationary hT.
  - The two output linear layers have no nonlinearity between them and are
    folded host-side into a single [256, 4] matmul + bias.
  - All weights ship as ONE packed DRAM blob -> one DMA -> one HWDGE queue
    semaphore, keeping per-instruction sync-wait counts within HW limits.
"""

import numpy as np

B, T, I, H, O = 256, 512, 64, 256, 4
NCORES = 8
BS = B // NCORES  # 32

# reference gate order is (i, f, g, o); we reorder to (i, f, o, g) so that the
# sigmoid-family gates are contiguous (cols 0:192) and the tanh gate is cols
# 192:256 of each 256-col group.
GATE_PERM = [0, 1, 3, 2]

# weight blob column offsets (bf16 elements, [128, WB_COLS])
OFF_W0 = 0        # Whh0 perm  [128, 2*1024]
OFF_W1 = 2048     # Whh1 perm  [128, 2*1024]
OFF_WX1 = 4096    # Wih1 perm  [128, 2*1024]
OFF_WX0 = 6144    # Wih0 perm + bias row, rows 0:65, [65, 1024]
OFF_B1 = 7168     # bias1 row, row 0, [1, 1024]
OFF_WF = 8192     # folded head weight [128, 2*4]
OFF_BF = 8200     # folded head bias, row 0, [1, 4]
OFF_XT = 8224     # x transposed + ones row, rows 0:65, [65, t_steps*32]
def _wb_cols(t_steps):
    return OFF_XT + t_steps * BS

_CACHED = {}


def _perm_cols(Wt):
    """Permute gate columns of [K, 1024] (col j = gate_orig*256 + h) into
    col = m*256 + gate_new*64 + hh*32 + hl, where h = hh*128 + m*32 + hl."""
    K = Wt.shape[0]
    W = Wt.reshape(K, 4, 256)[:, GATE_PERM, :]      # [K, gate, h]
    W = W.reshape(K, 4, 2, 4, 32)                    # [K, gate, hh, m, hl]
    W = W.transpose(0, 3, 1, 2, 4)                   # [K, m, gate, hh, hl]
    return np.ascontiguousarray(W.reshape(K, 1024), dtype=np.float32)


def _scale_gates(Wt):
    """Scale the i, f, o gate columns of [K, 1024] (ORIGINAL gate layout,
    col j = gate_orig*256 + h; orig order i,f,g,o) by 0.5 so that
    tanh(pre) == tanh(orig_pre/2) and sigma(orig_pre) = (tanh+1)/2."""
    s = np.ones(1024, np.float32)
    s[0:256] = 0.5      # i
    s[256:512] = 0.5    # f
    s[768:1024] = 0.5   # o
    return Wt * s[None, :]


def _build_bass(t_steps=T):
    import concourse.mybir as mybir
    import concourse.tile as tile
    from concourse import bacc
    from contextlib import ExitStack

    f32 = mybir.dt.float32
    bf16 = mybir.dt.bfloat16
    AF = mybir.ActivationFunctionType
    ALU = mybir.AluOpType

    nc = bacc.Bacc("TRN2", target_bir_lowering=False)

    wb_cols = _wb_cols(t_steps)
    wb_d = nc.dram_tensor("wb", (128, wb_cols), bf16, kind="ExternalInput")
    y_d = nc.dram_tensor("y", (BS, O), f32, kind="ExternalOutput")

    with tile.TileContext(nc) as tc, ExitStack() as ctx:
        const = ctx.enter_context(tc.tile_pool(name="const", bufs=1))
        state = ctx.enter_context(tc.tile_pool(name="state", bufs=1))
        work = ctx.enter_context(tc.tile_pool(name="work", bufs=3))
        hts = ctx.enter_context(tc.tile_pool(name="hts", bufs=4))
        psum = ctx.enter_context(tc.tile_pool(name="psum", bufs=3, space="PSUM"))

        wb = const.tile([128, wb_cols], bf16)
        nc.sync.dma_start(wb[:], wb_d[:])

        def xt_ap(t):
            return wb[0:65, OFF_XT + BS * t : OFF_XT + BS * t + BS]

        def w0_ap(kc, m):
            return wb[:, OFF_W0 + 1024 * kc + 256 * m : OFF_W0 + 1024 * kc + 256 * m + 256]

        def w1_ap(kc, m):
            return wb[:, OFF_W1 + 1024 * kc + 256 * m : OFF_W1 + 1024 * kc + 256 * m + 256]

        def wx1_ap(kc, m):
            return wb[:, OFF_WX1 + 1024 * kc + 256 * m : OFF_WX1 + 1024 * kc + 256 * m + 256]

        def wx0_ap(m):
            return wb[0:65, OFF_WX0 + 256 * m : OFF_WX0 + 256 * m + 256]

        def b1_ap(m):
            return wb[0:1, OFF_B1 + 256 * m : OFF_B1 + 256 * m + 256]

        cs0 = state.tile([128, 64], f32)
        cs1 = state.tile([128, 64], f32)
        nc.vector.memset(cs0[:], 0.0)
        nc.vector.memset(cs1[:], 0.0)
        hT0 = hts.tile([128, 64], bf16, tag="ht0")
        hT1 = hts.tile([128, 64], bf16, tag="ht1")
        nc.vector.memset(hT0[:], 0.0)
        nc.vector.memset(hT1[:], 0.0)
        ones_t = const.tile([1, BS], bf16)
        nc.vector.memset(ones_t[:], 1.0)
        ones_ap = ones_t[:]

        # HAM warm-up: sustained full-array matmuls (K=128, M=128, ~6us of
        # continuous PE activity while the wb DMA is still in flight) to flip
        # the PE clock gate from 4/8 (1.2 GHz) to 8/8 (2.4 GHz). The kernel's
        # steady-state PE gaps are well under the MID re-throttle window.
        warm_src = const.tile([128, 256], bf16)
        nc.vector.memset(warm_src[:], 0.0)
        warm_ps = psum.tile([128, 256], f32, tag="warm", bufs=1)
        for _ in range(28):
            nc.tensor.matmul(warm_ps[:], warm_src[:, 0:128], warm_src[:],
                             start=True, stop=True, skip_group_check=True)

        def elementwise(g, cs, tagsuf):
            # gate tile t holds tanh of (pre-scaled) gates: cols
            # 0:64=t_i, 64:128=t_f, 128:192=t_o, 192:256=t_g (per m-group)
            t = work.tile([128, 256], bf16, tag="t" + tagsuf)
            nc.scalar.activation(t[:], g[:], AF.Tanh)
            # A = (t_f + 1) * cs            ( = 4*sigma(f)*c )
            A = work.tile([128, 64], f32, tag="A" + tagsuf)
            nc.vector.scalar_tensor_tensor(
                A[:], t[:, 64:128], 1.0, cs[:], ALU.add, ALU.mult)
            # B = (t_i + 1) * t_g           ( = 2*sigma(i)*gtilde )
            Bt = work.tile([128, 64], bf16, tag="B" + tagsuf)
            nc.vector.scalar_tensor_tensor(
                Bt[:], t[:, 0:64], 1.0, t[:, 192:256], ALU.add, ALU.mult)
            # cs' = A*0.5 + B               ( = 2*c' )
            nc.vector.scalar_tensor_tensor(
                cs[:], A[:], 0.5, Bt[:], ALU.mult, ALU.add)
            # th = tanh(cs'/2) = tanh(c')
            th = work.tile([128, 64], bf16, tag="th" + tagsuf)
            nc.scalar.activation(th[:], cs[:], AF.Tanh, scale=0.5)
            # h2 = (t_o + 1) * th           ( = 2*h' )
            h2 = work.tile([128, 64], bf16, tag="h" + tagsuf)
            nc.vector.scalar_tensor_tensor(
                h2[:], t[:, 128:192], 1.0, th[:], ALU.add, ALU.mult)
            hT = hts.tile([128, 64], bf16, tag="ht" + tagsuf)
            nc.vector.transpose(hT[:], h2[:])
            return hT

        def grp(g, lhsT, w_ap_m, start, stop):
            for m in range(4):
                nc.tensor.matmul(
                    g[32 * m : 32 * m + 32, :], lhsT, w_ap_m(m),
                    start=start, stop=stop,
                    tile_position=(0, 32 * m), skip_group_check=True,
                )

        def step0(t, hT0_prev):
            g = psum.tile([128, 256], f32, tag="g0")
            grp(g, xt_ap(t), wx0_ap, True, False)
            for kc in range(2):
                grp(g, hT0_prev[:, 32 * kc : 32 * kc + 32],
                    lambda m, kc=kc: w0_ap(kc, m), False, kc == 1)
            return elementwise(g, cs0, "0")

        def step1(hT0_t, hT1_prev):
            g = psum.tile([128, 256], f32, tag="g1")
            grp(g, ones_ap, b1_ap, True, False)
            # hT1 (ready a full step earlier) before hT0 so the PE only
            # stalls on the chain-critical hT0 groups at the end.
            for kc in range(2):
                grp(g, hT1_prev[:, 32 * kc : 32 * kc + 32],
                    lambda m, kc=kc: w1_ap(kc, m), False, False)
            for kc in range(2):
                grp(g, hT0_t[:, 32 * kc : 32 * kc + 32],
                    lambda m, kc=kc: wx1_ap(kc, m), False, kc == 1)
            return elementwise(g, cs1, "1")

        hT0_hist = [hT0]
        for t in range(t_steps):
            hT0_new = step0(t, hT0_hist[-1])
            hT0_hist.append(hT0_new)
            # layer 1 lags by one step so the two chains overlap
            if t >= 1:
                hT1 = step1(hT0_hist[-2], hT1)
            if len(hT0_hist) > 3:
                hT0_hist.pop(0)
        hT1 = step1(hT0_hist[-1], hT1)

        yp = psum.tile([BS, O], f32, tag="yh", bufs=1)
        nc.tensor.matmul(yp[:], ones_ap, wb[0:1, OFF_BF : OFF_BF + O], start=True, stop=False)
        nc.tensor.matmul(yp[:], hT1[:, 0:32], wb[:, OFF_WF : OFF_WF + O], start=False, stop=False)
        nc.tensor.matmul(yp[:], hT1[:, 32:64], wb[:, OFF_WF + O : OFF_WF + 2 * O], start=False, stop=True)
        y_sb = work.tile([BS, O], f32, tag="y")
        nc.vector.tensor_copy(y_sb[:], yp[:])
        nc.sync.dma_start(y_d[:], y_sb[:])

    return nc


def _prep_inputs(x, Wih0, Whh0, bih0, bhh0, Wih1, Whh1, bih1, bhh1, W1, b1, W2, b2,
                 t_steps=T):
    import ml_dtypes

    x = np.asarray(x, dtype=np.float32)[:, :t_steps, :]
    wb = np.zeros((128, _wb_cols(t_steps)), np.float32)
    # 0.5 on i,f,o gate cols (tanh-only activation); 0.5 on every weight row
    # that consumes h2 = 2h (Whh0, Whh1, Wih1, head Wf).
    wb[:, OFF_W0 : OFF_W0 + 2048] = _perm_cols(
        _scale_gates(np.asarray(Whh0, np.float32).T) * 0.5
    ).reshape(2, 128, 1024).transpose(1, 0, 2).reshape(128, 2048)
    wb[:, OFF_W1 : OFF_W1 + 2048] = _perm_cols(
        _scale_gates(np.asarray(Whh1, np.float32).T) * 0.5
    ).reshape(2, 128, 1024).transpose(1, 0, 2).reshape(128, 2048)
    wb[:, OFF_WX1 : OFF_WX1 + 2048] = _perm_cols(
        _scale_gates(np.asarray(Wih1, np.float32).T) * 0.5
    ).reshape(2, 128, 1024).transpose(1, 0, 2).reshape(128, 2048)
    wb[0:64, OFF_WX0 : OFF_WX0 + 1024] = _perm_cols(
        _scale_gates(np.asarray(Wih0, np.float32).T))
    wb[64, OFF_WX0 : OFF_WX0 + 1024] = _perm_cols(_scale_gates(
        (np.asarray(bih0, np.float32) + np.asarray(bhh0, np.float32))[None, :]))[0]
    wb[0, OFF_B1 : OFF_B1 + 1024] = _perm_cols(_scale_gates(
        (np.asarray(bih1, np.float32) + np.asarray(bhh1, np.float32))[None, :]))[0]
    Wf = (np.asarray(W1, np.float32).T @ np.asarray(W2, np.float32).T).astype(
        np.float32) * 0.5
    wb[:, OFF_WF : OFF_WF + 2 * O] = Wf.reshape(2, 128, O).transpose(1, 0, 2).reshape(128, 2 * O)
    wb[0, OFF_BF : OFF_BF + O] = (
        np.asarray(b1, np.float32) @ np.asarray(W2, np.float32).T + np.asarray(b2, np.float32))

    in_maps = []
    for c in range(NCORES):
        xc = x[c * BS : (c + 1) * BS]                       # [BS, t, I]
        xt = xc.transpose(2, 1, 0).reshape(I, t_steps * BS) # [I, t*BS]
        wbc = wb.copy()
        wbc[0:64, OFF_XT:] = xt
        wbc[64, OFF_XT:] = 1.0
        in_maps.append(dict(wb=wbc.astype(ml_dtypes.bfloat16)))
    return in_maps


def run(t_steps=T, trace=False, **inputs):
    from concourse.bass_utils import run_bass_kernel_spmd

    key = t_steps
    if key not in _CACHED:
        nc_new = _build_bass(t_steps)
        # finalize BEFORE handing to the PJRT path: the bass_exec lowering
        # otherwise finalizes with the partition-id register preamble in a
        # state that miscompiles (walrus "Reg has not been allocated yet")
        nc_new.finalize()
        _CACHED[key] = nc_new
    nc = _CACHED[key]
    in_maps = _prep_inputs(**inputs, t_steps=t_steps)
    res = None
    for attempt in range(4):
        try:
            res = run_bass_kernel_spmd(nc, in_maps, core_ids=list(range(NCORES)),
                                       trace=trace)
            break
        except Exception as e:  # flaky parallel-birverifier race in neuronx-cc
            if attempt == 3:
                raise
            print(f"run attempt {attempt} failed ({type(e).__name__}); retrying")
    assert res is not None
    y = np.concatenate([r["y"] for r in res.results], axis=0)
    return y, res


def kernel(**inputs):
    y, _ = run(t_steps=T, trace=False, **inputs)
    return y
